# revision 12
# baseline (speedup 1.0000x reference)
"""Trainium2 Bass kernel for the AESUELOGIT segment-reduce problem.

Strategy (8 NeuronCores, SPMD):
  - Shard the 20000 paths across cores ALIGNED TO OD BOUNDARIES (core i owns
    ods [500i, 500(i+1)) and all their paths).  The segmented softmax is then
    fully core-local: no denominator collective is needed.
  - Head: features arrive host-transposed (link-partition, bf16); the
    link-utility lhsT is built by DVE ops k-group-pipelined with the feature
    DMA.  All critical head compute lives on the vector queue, which carries
    no DMAs; D-block loads are contiguous block-major streams.
  - matmul1 is a SINGLE bf16 pass.  Per 128-path chunk: transpose -> exp
    (bias adds the theta_links row, written straight to bf16) -> segment-sum
    matmul -> denominator gather matmuls (stm + merged stitch) -> flows (stt
    straight to bf16) -> matmul2 accumulation over all chunks into 4 PSUM
    banks.
  - A tiny AllGather early in the run aligns the 8 cores so the final
    collective does not pay the NEFF start skew.  ONE bf16 ReduceScatter of
    the (96, 2048) partial link flows at the end; each core runs the BPR
    epilogue on its 12 day-hour rows in a folded (96, 256) layout; the host
    concatenates the 8 slices.
"""

import os

import numpy as np
import ml_dtypes

import concourse.bacc as bacc
import concourse.bass as bass
import concourse.mybir as mybir
import concourse.tile as tile
from concourse.bass_utils import run_bass_kernel_spmd

F32 = mybir.dt.float32
BF16 = mybir.dt.bfloat16
AF = mybir.ActivationFunctionType
ALU = mybir.AluOpType

ND, NH, NL, NF = 4, 24, 2000, 4
NP, NOD, NCORES = 20000, 4000, 8
DH = ND * NH            # 96
DHS = DH // NCORES      # 12 day-hour rows per core after ReduceScatter
L_PAD = 2048            # links padded to 16*128
KL = L_PAD // 128       # 16 link chunks
KG = 4                  # k-groups for the head pipeline (4 chunks each)
KPG = KL // KG
FB = L_PAD // 256       # 8 link blocks in the folded epilogue layout
EPS = 1e-12


def _build_program(PSHARD, SLOT):
    """Emit the SPMD Bass program (identical on all cores)."""
    NCH = PSHARD // 128
    NPS = (PSHARD + 511) // 512          # 512-path blocks
    SL1 = SLOT + 1                       # seg matmul emits asc slots + last-od
    nc = bacc.Bacc("TRN2", target_bir_lowering=False, debug=False,
                   num_devices=NCORES)

    # ---- parameters (per-core shards) ----
    # features transposed: [link%128, (kg, f, kk, dh)] bf16
    p_ft = nc.dram_tensor("ftt", [128, KG * NF * KPG * DH], BF16,
                          kind="ExternalInput")
    # D block-major: p_d[b][p, 512k + j] = D[128k+p, 512b+j] (contiguous)
    p_d = nc.dram_tensor("dkb", [NPS, 128, KL * 512], BF16,
                         kind="ExternalInput")
    p_dt = nc.dram_tensor("dtk", [NCH, 128, L_PAD], BF16, kind="ExternalInput")
    p_s = nc.dram_tensor("seg", [128, NCH * SL1], BF16, kind="ExternalInput")
    p_stm = nc.dram_tensor("stm", [SLOT, NCH * 128], BF16,
                           kind="ExternalInput")
    p_stf = nc.dram_tensor("stf", [1, NCH * 128], BF16, kind="ExternalInput")
    p_stb = nc.dram_tensor("stb", [1, NCH * 128], BF16, kind="ExternalInput")
    p_qs = nc.dram_tensor("qsp", [128, NCH], F32, kind="ExternalInput")
    p_th = nc.dram_tensor("th", [1, NF], F32, kind="ExternalInput")
    p_tl = nc.dram_tensor("tl", [128, KL], F32, kind="ExternalInput")
    p_id = nc.dram_tensor("idn", [128, 128], F32, kind="ExternalInput")
    # folded (96, 256) epilogue tensors (host-replicated layouts, no math)
    p_kb = nc.dram_tensor("kb96", [DH, 256], F32, kind="ExternalInput")
    p_bb = nc.dram_tensor("bb96", [DH, 256], F32, kind="ExternalInput")
    p_lab = nc.dram_tensor("lab96", [DH, 256], F32, kind="ExternalInput")
    p_ttf = nc.dram_tensor("ttf96", [DH, 256], F32, kind="ExternalInput")
    p_out = nc.dram_tensor("out", [DH, 256], F32, kind="ExternalOutput")

    with tile.TileContext(nc) as tc:
        with tc.tile_pool(name="const", bufs=1) as cpool, \
             tc.tile_pool(name="dram", bufs=1, space="DRAM") as dpool, \
             tc.tile_pool(name="big", bufs=1) as bpool, \
             tc.tile_pool(name="stream", bufs=4) as spool, \
             tc.tile_pool(name="dkp", bufs=3) as dkpool, \
             tc.tile_pool(name="dtp", bufs=6) as dtpool:

            # ---- sync ring: th first (gates thb on vector), then features
            # (kg-major, pipelines with the v0t build), then D blocks 1..4
            th_sb = cpool.tile([1, NF], F32, tag="th")
            nc.sync.dma_start(th_sb[:], p_th.ap())
            tl_sb = cpool.tile([128, KL], F32, tag="tl")
            nc.sync.dma_start(tl_sb[:], p_tl.ap())
            GW = NF * KPG * DH               # cols per k-group in p_ft
            fts = bpool.tile([128, KG * GW], BF16, tag="fts")
            for g in range(KG):
                nc.sync.dma_start(fts[:, g * GW:(g + 1) * GW],
                                  p_ft.ap()[:, g * GW:(g + 1) * GW])
            ident = cpool.tile([128, 128], F32, tag="ident")
            nc.sync.dma_start(ident[:], p_id.ap())

            dk_t = [dkpool.tile([128, KL * 512], BF16, tag="dk",
                                name=f"dk{b}") for b in range(NPS)]

            def load_dk(b, eng):
                eng.dma_start(dk_t[b][:], p_d.ap()[b])

            # scalar ring carries ONLY dk0 before its per-chunk compute
            load_dk(0, nc.scalar)
            for b in range(1, NPS):
                load_dk(b, nc.sync)

            # gpsimd (SWDGE): skew-sync dummy AllGather, chunk-op constants,
            # first three D^T chunks, epilogue constants
            dum_sb = cpool.tile([1, 16], F32, tag="dum")
            nc.vector.memset(dum_sb[:], 1.0)
            dum_in = dpool.tile([1, 16], F32, tag="dumin")
            dum_out = dpool.tile([1, 16 * NCORES], F32, tag="dumout")
            nc.gpsimd.dma_start(dum_in[:], dum_sb[:])
            nc.gpsimd.collective_compute(
                "AllGather", ALU.bypass,
                replica_groups=[list(range(NCORES))],
                ins=[dum_in.opt()], outs=[dum_out.opt()])
            s_all = cpool.tile([128, NCH * SL1], BF16, tag="s_all")
            nc.gpsimd.dma_start(s_all[:], p_s.ap())
            stm_all = cpool.tile([SLOT, NCH * 128], BF16, tag="stm_all")
            nc.gpsimd.dma_start(stm_all[:], p_stm.ap())
            stf_all = cpool.tile([1, NCH * 128], BF16, tag="stf_all")
            nc.gpsimd.dma_start(stf_all[:], p_stf.ap())
            stb_all = cpool.tile([1, NCH * 128], BF16, tag="stb_all")
            nc.gpsimd.dma_start(stb_all[:], p_stb.ap())
            qs_sb = cpool.tile([128, NCH], F32, tag="qs")
            nc.gpsimd.dma_start(qs_sb[:], p_qs.ap())

            dt_t = [dtpool.tile([128, L_PAD], BF16, tag="dt",
                                name=f"dt{c}") for c in range(NCH)]
            for c in range(min(3, NCH)):
                nc.gpsimd.dma_start(dt_t[c][:], p_dt.ap()[c])

            # pull the skew-sync result onto SBUF; a vector op below waits on
            # it so all cores align before the block pipeline
            dumg = cpool.tile([1, 16 * NCORES], F32, tag="dumg")
            nc.gpsimd.dma_start(dumg[:], dum_out[:])

            kb = cpool.tile([DH, 256], F32, tag="kb")
            nc.gpsimd.dma_start(kb[:], p_kb.ap())
            bb = cpool.tile([DH, 256], F32, tag="bb")
            nc.gpsimd.dma_start(bb[:], p_bb.ap())
            lab = cpool.tile([DH, 256], F32, tag="lab")
            nc.gpsimd.dma_start(lab[:], p_lab.ap())
            ttf = cpool.tile([DH, 256], F32, tag="ttf")
            nc.gpsimd.dma_start(ttf[:], p_ttf.ap())

            thc = cpool.tile([1, NF], F32, tag="thc")
            nc.vector.tensor_scalar_min(thc[:], th_sb[:], 0.0)
            ones = cpool.tile([1, 128], F32, tag="ones")
            nc.vector.memset(ones[:], 1.0)

            with tc.tile_pool(name="psA", bufs=3, space="PSUM") as psA:
                thb_ps = psA.tile([128, NF], F32, tag="m")
                nc.tensor.matmul(thb_ps[:], ones[:], thc[:],
                                 start=True, stop=True)
                thb = cpool.tile([128, NF], F32, tag="thb")
                nc.vector.tensor_copy(thb[:], thb_ps[:])

                # lhsT build, k-group pipelined with the feature DMA:
                # v0t[l, (k, dh)] = sum_f theta_f * featsT, then packed to
                # lh[l, (k, dh..+tl)] with theta_links in col 96 of each chunk
                v0t = bpool.tile([128, KL * DH], F32, tag="v0t")
                lh = bpool.tile([128, KL * 97], BF16, tag="lh")
                GV = KPG * DH                # v0t cols per k-group
                for g in range(KG):
                    base = g * GW
                    nc.vector.tensor_scalar_mul(
                        v0t[:, g * GV:(g + 1) * GV],
                        fts[:, base:base + GV], thb[:, 0:1])
                    for f in range(1, NF):
                        nc.vector.scalar_tensor_tensor(
                            v0t[:, g * GV:(g + 1) * GV],
                            fts[:, base + f * GV:base + (f + 1) * GV],
                            thb[:, f:f + 1],
                            v0t[:, g * GV:(g + 1) * GV], ALU.mult, ALU.add)
                    lh3 = lh[:].rearrange("p (k j) -> p k j", j=97)
                    nc.vector.tensor_copy(
                        lh3[:, g * KPG:(g + 1) * KPG, 0:DH],
                        v0t[:, g * GV:(g + 1) * GV].rearrange(
                            "p (k j) -> p k j", j=DH))
                    nc.vector.tensor_copy(
                        lh3[:, g * KPG:(g + 1) * KPG, DH:DH + 1],
                        tl_sb[:, g * KPG:(g + 1) * KPG].rearrange(
                            "p (k j) -> p k j", j=1))

                # ---- the block pipeline ----
                ysb = bpool.tile([97, PSHARD], F32, tag="ysb")
                evb = bpool.tile([128, DH * NCH], BF16, tag="evb")
                tall = bpool.tile([SL1, DH * NCH], BF16, tag="tall")
                ft_bf = bpool.tile([128, DH * NCH], BF16, tag="ftb")
                qsq = cpool.tile([128, NCH], F32, tag="qsq")
                nc.vector.tensor_mul(qsq[:], qs_sb[:], qs_sb[:])
                # skew-sync: stalls fast cores here (overlapped w/ slow cores'
                # head) so the final collective starts aligned
                dumw = cpool.tile([1, 16 * NCORES], F32, tag="dumw")
                nc.vector.tensor_copy(dumw[:], dumg[:])
                ar_in = dpool.tile([DH, L_PAD], BF16, tag="arin")
                ar_out = dpool.tile([DHS, L_PAD], BF16, tag="arout")

                def chunk_softmax(c):
                    """transpose -> exp -> segment sums for path chunk c."""
                    if c + 3 < NCH:
                        nc.scalar.dma_start(dt_t[c + 3][:], p_dt.ap()[c + 3])
                    yt_ps = psA.tile([128, 97], F32, tag="m",
                                     name=f"yt{c}")
                    nc.tensor.matmul(yt_ps[:], ysb[:, 128 * c:128 * (c + 1)],
                                     ident[:97, :97], is_transpose=True,
                                     start=True, stop=True)
                    cvec = spool.tile([128, 1], F32, tag="cvec")
                    nc.scalar.copy(cvec[:], yt_ps[:, DH:DH + 1])
                    nc.scalar.activation(evb[:, DH * c:DH * (c + 1)],
                                         yt_ps[:, 0:DH], AF.Exp, bias=cvec[:])
                    ts_ps = psA.tile([SL1, DH], F32, tag="m",
                                     name=f"seg{c}")
                    nc.tensor.matmul(ts_ps[:],
                                     s_all[:, SL1 * c:SL1 * (c + 1)],
                                     evb[:, DH * c:DH * (c + 1)],
                                     start=True, stop=True)
                    nc.vector.tensor_copy(tall[:, DH * c:DH * (c + 1)],
                                          ts_ps[:])

                def chunk_flow(c):
                    """denominator gather + path flows + matmul2 for chunk c
                    (requires chunk c+1's segment sums, except the last)."""
                    g_ps = psA.tile([128, DH], F32, tag="m", name=f"g{c}")
                    cn = (c + 1) % NCH
                    cp = (c - 1) % NCH
                    nc.tensor.matmul(g_ps[:],
                                     stm_all[:, 128 * c:128 * (c + 1)],
                                     tall[0:SLOT, DH * c:DH * (c + 1)],
                                     start=True, stop=False)
                    nc.tensor.matmul(g_ps[:],
                                     stf_all[:, 128 * c:128 * (c + 1)],
                                     tall[0:1, DH * cn:DH * cn + DH],
                                     start=False, stop=False)
                    # row SLOT of chunk cp = its last-od partial; bounce it
                    # through a base-0 tile so the matmul base partitions match
                    tl_b = spool.tile([1, DH], BF16, tag="tlb")
                    nc.scalar.copy(tl_b[:],
                                   tall[SLOT:SL1, DH * cp:DH * cp + DH])
                    nc.tensor.matmul(g_ps[:],
                                     stb_all[:, 128 * c:128 * (c + 1)],
                                     tl_b[:], start=False, stop=True)
                    rec = spool.tile([128, DH], F32, tag="rec")
                    nc.vector.tensor_scalar_max(rec[:], g_ps[:], 1e-30)
                    nc.vector.reciprocal(rec[:], rec[:])
                    nc.vector.scalar_tensor_tensor(
                        ft_bf[:, DH * c:DH * (c + 1)],
                        evb[:, DH * c:DH * (c + 1)],
                        qsq[:, c:c + 1], rec[:], ALU.mult, ALU.mult)
                    for n in range(L_PAD // 512):
                        nc.tensor.matmul(
                            x_ps[n][:], ft_bf[:, DH * c:DH * (c + 1)],
                            dt_t[c][:, 512 * n:512 * (n + 1)],
                            start=(c == 0), stop=(c == NCH - 1))

                with tc.tile_pool(name="psV", bufs=1, space="PSUM") as psV, \
                     tc.tile_pool(name="psX", bufs=1, space="PSUM") as psX:
                    x_ps = [psX.tile([DH, 512], F32, tag=f"x{n}",
                                     name=f"x{n}")
                            for n in range(L_PAD // 512)]
                    for b in range(NPS):
                        w = min(512, PSHARD - 512 * b)
                        vf_ps = psV.tile([97, w], F32, tag="vf",
                                         name=f"vf{b}")
                        for k in range(KL):
                            nc.tensor.matmul(
                                vf_ps[:], lh[:, 97 * k:97 * (k + 1)],
                                dk_t[b][:, 512 * k:512 * k + w],
                                start=(k == 0), stop=(k == KL - 1))
                        nc.scalar.copy(ysb[:, 512 * b:512 * b + w],
                                       vf_ps[:])
                        for c in range(4 * b, min(4 * b + 4, NCH)):
                            chunk_softmax(c)
                            if c >= 1:
                                chunk_flow(c - 1)
                    chunk_flow(NCH - 1)

                    # drain: PSUM -> bf16 -> DRAM -> one ReduceScatter
                    xb = bpool.tile([DH, L_PAD], BF16, tag="xb")
                    for n in range(L_PAD // 512):
                        if n % 2 == 0:
                            nc.scalar.copy(xb[:, 512 * n:512 * (n + 1)],
                                           x_ps[n][:])
                        else:
                            nc.vector.tensor_copy(
                                xb[:, 512 * n:512 * (n + 1)], x_ps[n][:])
                        nc.sync.dma_start(
                            ar_in[:, 512 * n:512 * (n + 1)],
                            xb[:, 512 * n:512 * (n + 1)])
                    nc.gpsimd.collective_compute(
                        "ReduceScatter", ALU.add,
                        replica_groups=[list(range(NCORES))],
                        ins=[ar_in.opt()], outs=[ar_out.opt()])

                # ---- BPR epilogue in the folded (96, 256) layout ----
                ib = cpool.tile([DH, 256], F32, tag="ib")
                nc.vector.reciprocal(ib[:], kb[:])
                bb2 = cpool.tile([DH, 256], F32, tag="bb2")
                nc.vector.tensor_scalar(bb2[:], bb[:], float(EPS), 4.0,
                                        ALU.max, ALU.min)
                ab = cpool.tile([DH, 256], F32, tag="ab")
                nc.scalar.activation(ab[:], lab[:], AF.Exp)
                atf = cpool.tile([DH, 256], F32, tag="atf")
                nc.vector.tensor_mul(atf[:], ab[:], ttf[:])
                xg = bpool.tile([DH, 256], BF16, tag="xg")
                nc.sync.dma_start(
                    xg[:], ar_out.rearrange("d (a l) -> (d a) l", a=FB))
                t0 = bpool.tile([DH, 256], F32, tag="t0")
                nc.vector.tensor_mul(t0[:], xg[:], ib[:])
                nc.vector.tensor_scalar_max(t0[:], t0[:], 1e-35)
                t1 = bpool.tile([DH, 256], F32, tag="t1")
                nc.scalar.activation(t1[:], t0[:], AF.Ln)
                nc.vector.tensor_mul(t1[:], t1[:], bb2[:])
                t2 = bpool.tile([DH, 256], F32, tag="t2")
                nc.scalar.activation(t2[:], t1[:], AF.Exp)
                nc.vector.tensor_mul(t2[:], t2[:], atf[:])
                o_t = bpool.tile([DH, 256], F32, tag="o")
                nc.vector.tensor_add(o_t[:], t2[:], ttf[:])
                nc.sync.dma_start(p_out.ap(), o_t[:])

    nc.compile()
    return nc


_CACHE = {}
LAST_RESULT = None


def _get_program(PSHARD, SLOT):
    key = (PSHARD, SLOT)
    if key not in _CACHE:
        _CACHE[key] = _build_program(PSHARD, SLOT)
    return _CACHE[key]


def _fold96(v_lpad):
    """(L_PAD,) per-link vector -> (96, 256) folded layout (row 8*d + a holds
    link block [256a, 256(a+1)) for every local day-hour d)."""
    return np.ascontiguousarray(
        np.tile(v_lpad.reshape(FB, 256), (DHS, 1)).astype(np.float32))


def kernel(X, theta_raw, theta_links, q_sqrt, log_alpha, beta_raw, k, D,
           od_of_path, n_ods):
    X = np.asarray(X, np.float32)
    D = np.asarray(D, np.float32)
    od = np.asarray(od_of_path, np.int32)
    assert X.shape == (ND, NH, NL, NF + 1) and D.shape == (NL, NP)
    assert int(n_ods) == NOD

    od_per_core = (NOD + NCORES - 1) // NCORES
    bounds = np.searchsorted(od, np.arange(0, NOD + 1, od_per_core)[:NCORES + 1])
    bounds[0], bounds[-1] = 0, NP
    cnts = np.diff(bounds)
    PSHARD = int(np.ceil(cnts.max() / 128) * 128)
    NCH = PSHARD // 128
    NPS = (PSHARD + 511) // 512

    max_span = 1
    for i in range(NCORES):
        odl = od[bounds[i]:bounds[i + 1]]
        for c in range(0, len(odl), 128):
            ch = odl[c:c + 128]
            if len(ch):
                max_span = max(max_span, int(ch[-1] - ch[0]) + 1)
    W = int(np.ceil(max_span / 32) * 32)
    SLOT = W
    SL1 = SLOT + 1

    nc = _get_program(PSHARD, SLOT)

    # ---- host-side shard construction (index bookkeeping + relayout only) --
    Xf = X.reshape(DH, NL, NF + 1)
    ttf_full = np.zeros((DH, L_PAD), np.float32)
    ttf_full[:, :NL] = Xf[:, :, 0]
    # featsT[link%128, (kg, f, kk, dh)] bf16
    ftt = np.zeros((L_PAD, NF, DH), np.float32)
    for f in range(NF):
        ftt[:NL, f, :] = Xf[:, :, f + 1].T
    ftt = (ftt.reshape(KG, KPG, 128, NF, DH).transpose(2, 0, 3, 1, 4)
           .reshape(128, KG * NF * KPG * DH))
    ftt_h = np.ascontiguousarray(ftt).astype(ml_dtypes.bfloat16)

    def padded_vec(v, fill=0.0):
        o = np.full(L_PAD, fill, np.float32)
        o[:NL] = v
        return o

    tl_h = np.ascontiguousarray(
        padded_vec(np.asarray(theta_links, np.float32)).reshape(KL, 128).T)
    kb_h = _fold96(padded_vec(np.asarray(k, np.float32), fill=1.0))
    bb_h = _fold96(padded_vec(np.asarray(beta_raw, np.float32)))
    lab_h = _fold96(padded_vec(np.asarray(log_alpha, np.float32)))
    th_h = np.asarray(theta_raw, np.float32).reshape(1, NF)
    qsr = np.asarray(q_sqrt, np.float32)
    id_h = np.eye(128, dtype=np.float32)

    in_maps = []
    for i in range(NCORES):
        lo, hi = bounds[i], bounds[i + 1]
        cnt = hi - lo
        odl = od[lo:hi]

        PB = NPS * 512
        Dsh = np.zeros((L_PAD, PB), np.float32)
        Dsh[:NL, :cnt] = D[:, lo:hi]
        # block-major D: dkb[b][p, 512k + j] = D[128k+p, 512b+j]
        dkb = np.ascontiguousarray(
            Dsh.reshape(KL, 128, NPS, 512).transpose(2, 1, 0, 3)
            .reshape(NPS, 128, KL * 512)).astype(ml_dtypes.bfloat16)
        dt_h = np.ascontiguousarray(Dsh.T[:PSHARD]).astype(
            ml_dtypes.bfloat16).reshape(NCH, 128, L_PAD)

        s_h = np.zeros((128, NCH, SL1), ml_dtypes.bfloat16)
        stm_h = np.zeros((SLOT, NCH, 128), ml_dtypes.bfloat16)
        stf_h = np.zeros((1, NCH, 128), ml_dtypes.bfloat16)
        stb_h = np.zeros((1, NCH, 128), ml_dtypes.bfloat16)
        qs_h = np.zeros(PSHARD, np.float32)
        qs_h[:cnt] = qsr[odl]
        qs_h = np.ascontiguousarray(qs_h.reshape(NCH, 128).T)

        firsts, lasts = {}, {}
        for c in range(NCH):
            ch = odl[128 * c:128 * (c + 1)]
            if len(ch):
                firsts[c], lasts[c] = int(ch[0]), int(ch[-1])
        for c in range(NCH):
            ch = odl[128 * c:128 * (c + 1)]
            if not len(ch):
                continue
            f0, l0 = firsts[c], lasts[c]
            asc = ch - f0
            rows = np.arange(len(ch))
            s_h[rows, c, asc] = 1.0
            s_h[rows[ch == l0], c, SLOT] = 1.0   # last-od partial row
            stm_h[asc, c, rows] = 1.0
            if c + 1 in firsts and firsts[c + 1] == l0:
                stf_h[0, c, rows[ch == l0]] = 1.0
            if c - 1 in lasts and lasts[c - 1] == f0:
                stb_h[0, c, rows[ch == f0]] = 1.0

        in_maps.append(dict(
            ftt=ftt_h, dkb=dkb, dtk=dt_h,
            seg=np.ascontiguousarray(s_h.reshape(128, NCH * SL1)),
            stm=np.ascontiguousarray(stm_h.reshape(SLOT, NCH * 128)),
            stf=np.ascontiguousarray(stf_h.reshape(1, NCH * 128)),
            stb=np.ascontiguousarray(stb_h.reshape(1, NCH * 128)),
            qsp=qs_h, th=th_h, tl=tl_h, idn=id_h,
            kb96=kb_h, bb96=bb_h, lab96=lab_h,
            ttf96=np.ascontiguousarray(
                ttf_full[DHS * i:DHS * (i + 1)].reshape(DH, 256))))

    trace = os.environ.get("BASS_KERNEL_TRACE", "0") == "1"
    global LAST_RESULT
    for _attempt in range(3):
        res = run_bass_kernel_spmd(nc, in_maps, core_ids=list(range(NCORES)),
                                   trace=trace)
        LAST_RESULT = res
        parts = [r["out"].reshape(DHS, L_PAD) for r in res.results]
        out = np.concatenate(parts, axis=0)[:, :NL]
        if np.isfinite(out).all():
            break
    return np.ascontiguousarray(out).reshape(ND, NH, NL).astype(np.float32)


# revision 16
# speedup vs baseline: 1.5400x; 1.5400x over previous
"""Trainium2 Bass kernel for the AESUELOGIT segment-reduce problem.

Strategy (8 NeuronCores, SPMD):
  - Shard the 20000 paths across cores ALIGNED TO OD BOUNDARIES (core i owns
    ods [500i, 500(i+1)) and all their paths).  The segmented softmax is then
    fully core-local: no denominator collective is needed.
  - Head: features arrive host-transposed (link-partition, bf16); the
    link-utility lhsT is built by DVE ops k-group-pipelined with the feature
    DMA.  All critical head compute lives on the vector queue, which carries
    no DMAs.
  - matmul1: bf16 utilities against an fp8 D (0/1 incidence is exact in
    e4m3).  Per 128-path chunk: transpose -> exp (bias adds the theta_links
    row, written straight to bf16) -> denominators via host-built same-od
    0/1 matmuls (B within chunk + narrow X/Y stitches across the chunk
    boundary) -> flows (stt straight to fp8) -> matmul2 as fp8 DoubleRow
    over chunk PAIRS into 4 PSUM banks.
  - gpsimd carries only the collectives: a tiny AllGather issued at t=0
    aligns the CC engines, then ONE bf16 ReduceScatter of the (96, 2048)
    partial link flows at the end.  Each core runs the BPR epilogue on its
    12 day-hour rows in a folded (96, 256) layout; the host concatenates.
"""

import os

import numpy as np
import ml_dtypes

import concourse.bacc as bacc
import concourse.bass as bass
import concourse.mybir as mybir
import concourse.tile as tile
from concourse.bass_utils import run_bass_kernel_spmd

F32 = mybir.dt.float32
BF16 = mybir.dt.bfloat16
F8 = mybir.dt.float8e4
AF = mybir.ActivationFunctionType
ALU = mybir.AluOpType
DR = mybir.MatmulPerfMode.DoubleRow

ND, NH, NL, NF = 4, 24, 2000, 4
NP, NOD, NCORES = 20000, 4000, 8
DH = ND * NH            # 96
DHS = DH // NCORES      # 12 day-hour rows per core after ReduceScatter
L_PAD = 2048            # links padded to 16*128
KL = L_PAD // 128       # 16 link chunks
KG = 4                  # k-groups for the head pipeline (4 chunks each)
KPG = KL // KG
FB = L_PAD // 256       # 8 link blocks in the folded epilogue layout
EPS = 1e-12


def _build_program(PSHARD, XW):
    """Emit the SPMD Bass program (identical on all cores)."""
    NCH = PSHARD // 128                  # even (PSHARD rounded to 256)
    NPS = (PSHARD + 511) // 512          # 512-path blocks
    NPR = NCH // 2                       # chunk pairs for fp8 DoubleRow mm2
    nc = bacc.Bacc("TRN2", target_bir_lowering=False, debug=False,
                   num_devices=NCORES)

    # ---- parameters (per-core shards) ----
    # features transposed: [link%128, (kg, f, kk, dh)] bf16
    p_ft = nc.dram_tensor("ftt", [128, KG * NF * KPG * DH], BF16,
                          kind="ExternalInput")
    # D block-major fp8: p_d[b][p, 512k + j] = D[128k+p, 512b+j]
    p_d = nc.dram_tensor("dkb", [NPS, 128, KL * 512], F8,
                         kind="ExternalInput")
    # D^T chunk-pair-major fp8: p_dt[t][p, c*L_PAD + l], c in {0,1}
    p_dt = nc.dram_tensor("dtp", [NPR, 128, 2 * L_PAD], F8,
                          kind="ExternalInput")
    # same-od 0/1 matrices: B within chunk, X from next chunk's first XW
    # rows, Y from prev chunk's last XW rows (bounced to base 0)
    p_B = nc.dram_tensor("bod", [128, NCH * 128], BF16, kind="ExternalInput")
    p_X = nc.dram_tensor("xod", [XW, NCH * 128], BF16, kind="ExternalInput")
    p_Y = nc.dram_tensor("yod", [XW, NCH * 128], BF16, kind="ExternalInput")
    p_qs = nc.dram_tensor("qsp", [128, NCH], F32, kind="ExternalInput")
    p_th = nc.dram_tensor("th", [1, NF], F32, kind="ExternalInput")
    p_tl = nc.dram_tensor("tl", [128, KL], F32, kind="ExternalInput")
    p_id = nc.dram_tensor("idn", [128, 128], F32, kind="ExternalInput")
    # folded (96, 256) epilogue tensors (host-replicated layouts, no math)
    p_kb = nc.dram_tensor("kb96", [DH, 256], F32, kind="ExternalInput")
    p_bb = nc.dram_tensor("bb96", [DH, 256], F32, kind="ExternalInput")
    p_lab = nc.dram_tensor("lab96", [DH, 256], F32, kind="ExternalInput")
    p_ttf = nc.dram_tensor("ttf96", [DH, 256], F32, kind="ExternalInput")
    p_out = nc.dram_tensor("out", [DH, 256], F32, kind="ExternalOutput")

    with tile.TileContext(nc) as tc:
        with tc.tile_pool(name="const", bufs=1) as cpool, \
             tc.tile_pool(name="dram", bufs=1, space="DRAM") as dpool, \
             tc.tile_pool(name="big", bufs=1) as bpool, \
             tc.tile_pool(name="stream", bufs=4) as spool, \
             tc.tile_pool(name="dkp", bufs=3) as dkpool, \
             tc.tile_pool(name="dtp", bufs=4) as dtpool:

            # gpsimd carries ONLY collectives: the t=0 dummy AllGather keeps
            # the CC engines aligned for the final ReduceScatter.  Nothing
            # else may queue on gpsimd (a collective blocks its queue).
            dum_sb = cpool.tile([128, 4], F32, tag="dum")
            nc.vector.memset(dum_sb[:], 1.0)
            dum_in = dpool.tile([128, 4], F32, tag="dumin")
            dum_out = dpool.tile([128, 4 * NCORES], F32, tag="dumout")
            nc.gpsimd.dma_start(dum_in[:], dum_sb[:])
            nc.gpsimd.collective_compute(
                "AllGather", ALU.bypass,
                replica_groups=[list(range(NCORES))],
                ins=[dum_in.opt()], outs=[dum_out.opt()])

            # ---- sync ring: th first (gates thb on vector), features
            # (kg-major), ident, dk1, B/X/Y + qs, then dk/dt interleaved by
            # deadline, then the late epilogue constants
            th_sb = cpool.tile([1, NF], F32, tag="th")
            nc.sync.dma_start(th_sb[:], p_th.ap())
            tl_sb = cpool.tile([128, KL], F32, tag="tl")
            nc.sync.dma_start(tl_sb[:], p_tl.ap())
            GW = NF * KPG * DH               # cols per k-group in p_ft
            fts = bpool.tile([128, KG * GW], BF16, tag="fts")
            for g in range(KG):
                nc.sync.dma_start(fts[:, g * GW:(g + 1) * GW],
                                  p_ft.ap()[:, g * GW:(g + 1) * GW])
            ident = cpool.tile([128, 128], F32, tag="ident")
            nc.sync.dma_start(ident[:], p_id.ap())

            dk_t = [dkpool.tile([128, KL * 512], F8, tag="dk",
                                name=f"dk{b}") for b in range(NPS)]
            dtp_t = [dtpool.tile([128, 2 * L_PAD], F8, tag="dt",
                                 name=f"dt{t}") for t in range(NPR)]

            # scalar ring head: dk0 + first two dt pairs, then compute-only.
            # NOTE: pool-sharing DMAs must be emitted in buffer-rotation
            # order, so these precede the `later` sync loop.
            nc.scalar.dma_start(dk_t[0][:], p_d.ap()[0])
            if NPS > 1:
                nc.sync.dma_start(dk_t[1][:], p_d.ap()[1])
            for t in range(min(2, NPR)):
                nc.scalar.dma_start(dtp_t[t][:], p_dt.ap()[t])
            b_all = cpool.tile([128, NCH * 128], BF16, tag="b_all")
            nc.sync.dma_start(b_all[:], p_B.ap())
            x_all = cpool.tile([XW, NCH * 128], BF16, tag="x_all")
            nc.sync.dma_start(x_all[:], p_X.ap())
            y_all = cpool.tile([XW, NCH * 128], BF16, tag="y_all")
            nc.sync.dma_start(y_all[:], p_Y.ap())
            qs_sb = cpool.tile([128, NCH], F32, tag="qs")
            nc.sync.dma_start(qs_sb[:], p_qs.ap())
            # remaining dk blocks and dt pairs, ordered by use deadline
            later = [("dk", b) for b in range(2, NPS)]
            for t in range(2, NPR):
                later.insert(min(2 * (t - 2) + 1, len(later)), ("dt", t))
            for kind, i in later:
                if kind == "dk":
                    nc.sync.dma_start(dk_t[i][:], p_d.ap()[i])
                else:
                    nc.sync.dma_start(dtp_t[i][:], p_dt.ap()[i])
            kb = cpool.tile([DH, 256], F32, tag="kb")
            nc.sync.dma_start(kb[:], p_kb.ap())
            bb = cpool.tile([DH, 256], F32, tag="bb")
            nc.sync.dma_start(bb[:], p_bb.ap())
            lab = cpool.tile([DH, 256], F32, tag="lab")
            nc.sync.dma_start(lab[:], p_lab.ap())
            ttf = cpool.tile([DH, 256], F32, tag="ttf")
            nc.sync.dma_start(ttf[:], p_ttf.ap())

            thc = cpool.tile([1, NF], F32, tag="thc")
            nc.vector.tensor_scalar_min(thc[:], th_sb[:], 0.0)
            ones = cpool.tile([1, 128], F32, tag="ones")
            nc.vector.memset(ones[:], 1.0)

            with tc.tile_pool(name="psA", bufs=2, space="PSUM") as psA:
                thb_ps = psA.tile([128, NF], F32, tag="m")
                nc.tensor.matmul(thb_ps[:], ones[:], thc[:],
                                 start=True, stop=True)
                thb = cpool.tile([128, NF], F32, tag="thb")
                nc.vector.tensor_copy(thb[:], thb_ps[:])

                # lhsT build, k-group pipelined with the feature DMA:
                # v0t[l, (k, dh)] = sum_f theta_f * featsT, then packed to
                # lh[l, (k, dh..+tl)] with theta_links in col 96 of each chunk
                v0t = bpool.tile([128, KL * DH], F32, tag="v0t")
                lh = bpool.tile([128, KL * 97], BF16, tag="lh")
                GV = KPG * DH                # v0t cols per k-group
                for g in range(KG):
                    base = g * GW
                    nc.vector.tensor_scalar_mul(
                        v0t[:, g * GV:(g + 1) * GV],
                        fts[:, base:base + GV], thb[:, 0:1])
                    for f in range(1, NF):
                        nc.vector.scalar_tensor_tensor(
                            v0t[:, g * GV:(g + 1) * GV],
                            fts[:, base + f * GV:base + (f + 1) * GV],
                            thb[:, f:f + 1],
                            v0t[:, g * GV:(g + 1) * GV], ALU.mult, ALU.add)
                    lh3 = lh[:].rearrange("p (k j) -> p k j", j=97)
                    nc.vector.tensor_copy(
                        lh3[:, g * KPG:(g + 1) * KPG, 0:DH],
                        v0t[:, g * GV:(g + 1) * GV].rearrange(
                            "p (k j) -> p k j", j=DH))
                    nc.vector.tensor_copy(
                        lh3[:, g * KPG:(g + 1) * KPG, DH:DH + 1],
                        tl_sb[:, g * KPG:(g + 1) * KPG].rearrange(
                            "p (k j) -> p k j", j=1))

                # ---- the block pipeline ----
                ysb = bpool.tile([97, PSHARD], F32, tag="ysb")
                evb = bpool.tile([128, DH * NCH], BF16, tag="evb")
                ft8 = bpool.tile([128, DH * NCH], F8, tag="ft8")
                qsq = cpool.tile([128, NCH], F32, tag="qsq")
                nc.vector.tensor_mul(qsq[:], qs_sb[:], qs_sb[:])
                ar_in = dpool.tile([DH, L_PAD], BF16, tag="arin")
                ar_out = dpool.tile([DHS, L_PAD], BF16, tag="arout")

                def chunk_softmax(c):
                    """transpose -> exp for path chunk c."""
                    yt_ps = psA.tile([128, 97], F32, tag="m",
                                     name=f"yt{c}")
                    nc.tensor.matmul(yt_ps[:], ysb[:, 128 * c:128 * (c + 1)],
                                     ident[:97, :97], is_transpose=True,
                                     start=True, stop=True)
                    cvec = spool.tile([128, 1], F32, tag="cvec")
                    nc.scalar.copy(cvec[:], yt_ps[:, DH:DH + 1])
                    nc.scalar.activation(evb[:, DH * c:DH * (c + 1)],
                                         yt_ps[:, 0:DH], AF.Exp, bias=cvec[:])

                def chunk_flow(c):
                    """same-od denominator matmuls + path flows for chunk c
                    (needs chunk c+1's exp values); fp8 DoubleRow matmul2 on
                    each odd c for the pair (c-1, c)."""
                    g_ps = psA.tile([128, DH], F32, tag="m", name=f"g{c}")
                    cn = (c + 1) % NCH
                    cp = (c - 1) % NCH
                    nc.tensor.matmul(g_ps[:],
                                     b_all[:, 128 * c:128 * (c + 1)],
                                     evb[:, DH * c:DH * (c + 1)],
                                     start=True, stop=False)
                    nc.tensor.matmul(g_ps[:],
                                     x_all[:, 128 * c:128 * (c + 1)],
                                     evb[0:XW, DH * cn:DH * cn + DH],
                                     start=False, stop=False)
                    # prev chunk's last XW rows, bounced to a base-0 tile
                    evt = spool.tile([XW, DH], BF16, tag="evt")
                    nc.scalar.copy(evt[:],
                                   evb[128 - XW:128, DH * cp:DH * cp + DH])
                    nc.tensor.matmul(g_ps[:],
                                     y_all[:, 128 * c:128 * (c + 1)],
                                     evt[:], start=False, stop=True)
                    rec = spool.tile([128, DH], F32, tag="rec")
                    nc.vector.tensor_scalar_max(rec[:], g_ps[:], 1e-30)
                    nc.vector.reciprocal(rec[:], rec[:])
                    nc.vector.scalar_tensor_tensor(
                        ft8[:, DH * c:DH * (c + 1)],
                        evb[:, DH * c:DH * (c + 1)],
                        qsq[:, c:c + 1], rec[:], ALU.mult, ALU.mult)
                    if c % 2 == 1:
                        t = c // 2
                        lhs3 = ft8[:].rearrange(
                            "p (c m) -> p c m", m=DH)[:, c - 1:c + 1, :]
                        rhs3 = dtp_t[t][:].rearrange(
                            "p (c l) -> p c l", c=2)
                        for n in range(L_PAD // 512):
                            nc.tensor.matmul(
                                x_ps[n][:], lhs3,
                                rhs3[:, :, 512 * n:512 * (n + 1)],
                                perf_mode=DR,
                                start=(c == 1), stop=(c == NCH - 1))

                with tc.tile_pool(name="psV", bufs=2, space="PSUM") as psV, \
                     tc.tile_pool(name="psX", bufs=1, space="PSUM") as psX:
                    x_ps = [psX.tile([DH, 512], F32, tag=f"x{n}",
                                     name=f"x{n}")
                            for n in range(L_PAD // 512)]
                    for b in range(NPS):
                        w = min(512, PSHARD - 512 * b)
                        vf_ps = psV.tile([97, w], F32, tag="vf",
                                         name=f"vf{b}")
                        for k in range(KL):
                            nc.tensor.matmul(
                                vf_ps[:], lh[:, 97 * k:97 * (k + 1)],
                                dk_t[b][:, 512 * k:512 * k + w],
                                start=(k == 0), stop=(k == KL - 1))
                        nc.scalar.copy(ysb[:, 512 * b:512 * b + w],
                                       vf_ps[:])
                        for c in range(4 * b, min(4 * b + 4, NCH)):
                            chunk_softmax(c)
                            if c >= 1:
                                chunk_flow(c - 1)
                    chunk_flow(NCH - 1)

                    # drain: PSUM -> bf16 -> DRAM -> one ReduceScatter
                    xb = bpool.tile([DH, L_PAD], BF16, tag="xb")
                    for n in range(L_PAD // 512):
                        if n % 2 == 0:
                            nc.scalar.copy(xb[:, 512 * n:512 * (n + 1)],
                                           x_ps[n][:])
                        else:
                            nc.vector.tensor_copy(
                                xb[:, 512 * n:512 * (n + 1)], x_ps[n][:])
                        nc.sync.dma_start(
                            ar_in[:, 512 * n:512 * (n + 1)],
                            xb[:, 512 * n:512 * (n + 1)])
                    nc.gpsimd.collective_compute(
                        "ReduceScatter", ALU.add,
                        replica_groups=[list(range(NCORES))],
                        ins=[ar_in.opt()], outs=[ar_out.opt()])

                # ---- BPR epilogue in the folded (96, 256) layout ----
                ib = cpool.tile([DH, 256], F32, tag="ib")
                nc.vector.reciprocal(ib[:], kb[:])
                bb2 = cpool.tile([DH, 256], F32, tag="bb2")
                nc.vector.tensor_scalar(bb2[:], bb[:], float(EPS), 4.0,
                                        ALU.max, ALU.min)
                ab = cpool.tile([DH, 256], F32, tag="ab")
                nc.scalar.activation(ab[:], lab[:], AF.Exp)
                atf = cpool.tile([DH, 256], F32, tag="atf")
                nc.vector.tensor_mul(atf[:], ab[:], ttf[:])
                xg = bpool.tile([DH, 256], BF16, tag="xg")
                nc.sync.dma_start(
                    xg[:], ar_out.rearrange("d (a l) -> (d a) l", a=FB))
                t0 = bpool.tile([DH, 256], F32, tag="t0")
                nc.vector.tensor_mul(t0[:], xg[:], ib[:])
                nc.vector.tensor_scalar_max(t0[:], t0[:], 1e-35)
                t1 = bpool.tile([DH, 256], F32, tag="t1")
                nc.scalar.activation(t1[:], t0[:], AF.Ln)
                nc.vector.tensor_mul(t1[:], t1[:], bb2[:])
                t2 = bpool.tile([DH, 256], F32, tag="t2")
                nc.scalar.activation(t2[:], t1[:], AF.Exp)
                nc.vector.tensor_mul(t2[:], t2[:], atf[:])
                o_t = bpool.tile([DH, 256], F32, tag="o")
                nc.vector.tensor_add(o_t[:], t2[:], ttf[:])
                nc.sync.dma_start(p_out.ap(), o_t[:])

    nc.compile()
    return nc


_CACHE = {}
LAST_RESULT = None


def _get_program(PSHARD, XW):
    key = (PSHARD, XW)
    if key not in _CACHE:
        _CACHE[key] = _build_program(PSHARD, XW)
    return _CACHE[key]


def _fold96(v_lpad):
    """(L_PAD,) per-link vector -> (96, 256) folded layout (row 8*d + a holds
    link block [256a, 256(a+1)) for every local day-hour d)."""
    return np.ascontiguousarray(
        np.tile(v_lpad.reshape(FB, 256), (DHS, 1)).astype(np.float32))


def kernel(X, theta_raw, theta_links, q_sqrt, log_alpha, beta_raw, k, D,
           od_of_path, n_ods):
    X = np.asarray(X, np.float32)
    D = np.asarray(D, np.float32)
    od = np.asarray(od_of_path, np.int32)
    assert X.shape == (ND, NH, NL, NF + 1) and D.shape == (NL, NP)
    assert int(n_ods) == NOD

    od_per_core = (NOD + NCORES - 1) // NCORES
    bounds = np.searchsorted(od, np.arange(0, NOD + 1, od_per_core)[:NCORES + 1])
    bounds[0], bounds[-1] = 0, NP
    cnts = np.diff(bounds)
    PSHARD = int(np.ceil(cnts.max() / 256) * 256)   # NCH even for mm2 pairs
    NCH = PSHARD // 128
    NPS = (PSHARD + 511) // 512

    # X/Y stitch width: enough rows to cover any single od's paths
    odc = np.bincount(od, minlength=NOD)
    XW = int(np.ceil(max(1, odc.max()) / 32) * 32)
    XW = min(XW, 128)

    nc = _get_program(PSHARD, XW)

    F8H = ml_dtypes.float8_e4m3fn

    # ---- host-side shard construction (index bookkeeping + relayout only) --
    Xf = X.reshape(DH, NL, NF + 1)
    ttf_full = np.zeros((DH, L_PAD), np.float32)
    ttf_full[:, :NL] = Xf[:, :, 0]
    # featsT[link%128, (kg, f, kk, dh)] bf16
    ftt = np.zeros((L_PAD, NF, DH), np.float32)
    for f in range(NF):
        ftt[:NL, f, :] = Xf[:, :, f + 1].T
    ftt = (ftt.reshape(KG, KPG, 128, NF, DH).transpose(2, 0, 3, 1, 4)
           .reshape(128, KG * NF * KPG * DH))
    ftt_h = np.ascontiguousarray(ftt).astype(ml_dtypes.bfloat16)

    def padded_vec(v, fill=0.0):
        o = np.full(L_PAD, fill, np.float32)
        o[:NL] = v
        return o

    tl_h = np.ascontiguousarray(
        padded_vec(np.asarray(theta_links, np.float32)).reshape(KL, 128).T)
    kb_h = _fold96(padded_vec(np.asarray(k, np.float32), fill=1.0))
    bb_h = _fold96(padded_vec(np.asarray(beta_raw, np.float32)))
    lab_h = _fold96(padded_vec(np.asarray(log_alpha, np.float32)))
    th_h = np.asarray(theta_raw, np.float32).reshape(1, NF)
    qsr = np.asarray(q_sqrt, np.float32)
    id_h = np.eye(128, dtype=np.float32)

    in_maps = []
    for i in range(NCORES):
        lo, hi = bounds[i], bounds[i + 1]
        cnt = hi - lo
        odl = od[lo:hi]

        PB = NPS * 512
        Dsh = np.zeros((L_PAD, PB), np.float32)
        Dsh[:NL, :cnt] = D[:, lo:hi]
        # block-major D fp8: dkb[b][p, 512k + j] = D[128k+p, 512b+j]
        dkb = np.ascontiguousarray(
            Dsh.reshape(KL, 128, NPS, 512).transpose(2, 1, 0, 3)
            .reshape(NPS, 128, KL * 512)).astype(F8H)
        # chunk-pair-major D^T fp8: dtp[t][p, c*L_PAD + l] = D^T[(2t+c)*128+p, l]
        dtp = np.ascontiguousarray(
            Dsh.T[:PSHARD].astype(F8H).reshape(NCH // 2, 2, 128, L_PAD)
            .transpose(0, 2, 1, 3).reshape(NCH // 2, 128, 2 * L_PAD))

        # same-od 0/1 matrices (pure index bookkeeping)
        odp = np.full(NCH * 128, -1, np.int64)
        odp[:cnt] = odl
        oc = odp.reshape(NCH, 128)
        b_h = np.zeros((128, NCH, 128), ml_dtypes.bfloat16)
        x_h = np.zeros((XW, NCH, 128), ml_dtypes.bfloat16)
        y_h = np.zeros((XW, NCH, 128), ml_dtypes.bfloat16)
        for c in range(NCH):
            b_h[:, c, :] = (oc[c][:, None] == oc[c][None, :])
            if c + 1 < NCH:
                x_h[:, c, :] = (oc[c + 1][:XW, None] == oc[c][None, :])
            if c > 0:
                y_h[:, c, :] = (oc[c - 1][128 - XW:, None] == oc[c][None, :])

        qs_h = np.zeros(PSHARD, np.float32)
        qs_h[:cnt] = qsr[odl]
        qs_h = np.ascontiguousarray(qs_h.reshape(NCH, 128).T)

        in_maps.append(dict(
            ftt=ftt_h, dkb=dkb, dtp=dtp,
            bod=np.ascontiguousarray(b_h.reshape(128, NCH * 128)),
            xod=np.ascontiguousarray(x_h.reshape(XW, NCH * 128)),
            yod=np.ascontiguousarray(y_h.reshape(XW, NCH * 128)),
            qsp=qs_h, th=th_h, tl=tl_h, idn=id_h,
            kb96=kb_h, bb96=bb_h, lab96=lab_h,
            ttf96=np.ascontiguousarray(
                ttf_full[DHS * i:DHS * (i + 1)].reshape(DH, 256))))

    trace = os.environ.get("BASS_KERNEL_TRACE", "0") == "1"
    global LAST_RESULT
    for _attempt in range(3):
        res = run_bass_kernel_spmd(nc, in_maps, core_ids=list(range(NCORES)),
                                   trace=trace)
        LAST_RESULT = res
        parts = [r["out"].reshape(DHS, L_PAD) for r in res.results]
        out = np.concatenate(parts, axis=0)[:, :NL]
        if np.isfinite(out).all():
            break
    return np.ascontiguousarray(out).reshape(ND, NH, NL).astype(np.float32)
